# revision 18
# baseline (speedup 1.0000x reference)
"""AttnBlock (GroupNorm + 4-head hd-64 self-attention + proj + residual)
Trainium2 Bass kernel, 8 NeuronCores.

Sharding: core i handles batch b = i//2 and head-pair hp = i%2 (heads 2hp, 2hp+1).
Each core computes GroupNorm stats for its batch (folded into the QKV GEMMs as a
per-channel affine on the weights/bias), runs flash-style attention for its two
heads on-chip, and emits partial[o, pix] = sum_{c in its 128 ch} w_proj[o,c]*attn.
Host: out[b] = x[b] + b_proj + sum_hp(partial[hp]/128 + w_proj[:,hp]@bv[hp]).

Structure (the kernel is ACT/DVE-bound: every S element must exit PSUM through
one of the two engines that can read PSUM):
- mm1 (QK^T, f32r) is ROW-TILED: head0 occupies PE rows 0-63, head1 rows 64-127
  (contraction is only hd=64), so both heads' matmuls run CONCURRENTLY in the
  array -> one [128kpix, 2(head), 512q] PSUM tile per 216ns window, and the PE
  cost of mm1 halves vs zero-padded k. No kz padding tiles needed.
- V is produced PRE-TRANSPOSED by the QKV GEMM itself: lhsT = x-chunk
  (stationary), rhs = w_v^T -> out[pix, (h,hd)] accumulates in PSUM; one fp8
  exit per head per 4-chunk bank writes vT8 directly. No PE-transpose pass,
  no v_sb, half the exits.
- Softmax exp splits across ACT (true exp via table) and DVE (Schraudolph:
  round(S'+24) bit pattern as fp8e4m3, computing 2^((S'-32)/8)); log2e is
  pre-folded into the Q weights on the host so both paths are 1 op.
- Biases: K-bias dropped entirely (adds a per-query constant to S -> softmax
  invariant). V-bias returned to host (softmax weights sum to 1, so it adds
  W_proj@bv to the output). Q-bias folded into the Q-exit epilogue.
- mm2 (attn@V) and proj run fp8 DoubleRow; denominator comes out of mm2 via an
  extra 1/32 ones-column in vT8; normalize (reciprocal+broadcast+mult) is on
  DVE/gpsimd off the exit-engine critical path; proj is pipelined per-qi.
"""

import numpy as np
import ml_dtypes

B, C, H, W = 4, 256, 64, 64
HW = H * W            # 4096 pixels
NH = 4                # heads
HD = 64               # head dim
NG = 8                # groupnorm groups
EPS = 1e-5
NCORES = 8

LOG2E = 1.4426950408889634
LN2 = 0.6931471805599453
B_SCH = 24.0                      # schraudolph bias: bits = round(S' + B)
# S' = log2e*S_raw (log2e folded into Q weights).  max raw S = 62.7 ->
# S' = 90.5 -> max bits 114 < 120 (fp8e4 inf); low tail clamps to 0.
BETA_ACT = (B_SCH - 56.0) / 8.0 * LN2   # ACT path: exp(S'*ln2/8 + beta)
VSCALE = 32.0                     # denominator ones col = 1/32 -> attn x32
WSCALE = 4.0                      # w_proj stored x4
# exp engine split: per 32 chunk-blocks of a qi, how many go to ACT (rest DVE).
EXP_SPLIT = 17

_CACHE = {}


def _build(repeats=1, ablate="", unroll=False):
    import concourse.tile as tile
    from concourse import bacc, mybir

    f32 = mybir.dt.float32
    f8 = mybir.dt.float8e4

    nc = bacc.Bacc("TRN2", target_bir_lowering=False, debug=False,
                   enable_asserts=False, num_devices=NCORES)

    xb_d = nc.dram_tensor("xb", [256, HW], mybir.dt.float32r,
                          kind="ExternalInput").ap()
    wq_d = nc.dram_tensor("wq", [256, 256], f32, kind="ExternalInput").ap()   # [c, o] lhsT; o = q|k blocks of 128 (q cols pre-scaled by log2e)
    wv_d = nc.dram_tensor("wv", [256, 128], f32, kind="ExternalInput").ap()   # [c, (h,hd)] rhs for transposed V GEMM
    bq_d = nc.dram_tensor("bq", [2, 128, 1], f32, kind="ExternalInput").ap()  # [0]=q bias (x log2e), [1]=v bias
    wp_d = nc.dram_tensor("wp8", [64, 2, 256], f8, kind="ExternalInput").ap() # [r, h, o] x4
    gam_d = nc.dram_tensor("gam", [2, 128, 1], f32, kind="ExternalInput").ap()
    bet_d = nc.dram_tensor("bet", [2, 128, 1], f32, kind="ExternalInput").ap()
    sel_d = nc.dram_tensor("selc", [128, 4], f32, kind="ExternalInput").ap()
    selT_d = nc.dram_tensor("selT", [4, 128], f32, kind="ExternalInput").ap()
    vones_d = nc.dram_tensor("vones", [128, 32, 2], f8, kind="ExternalInput").ap()
    part_d = nc.dram_tensor("part", [256, HW], f32, kind="ExternalOutput").ap()
    bv_d = nc.dram_tensor("bv", [128, 1], f32, kind="ExternalOutput").ap()

    with tile.TileContext(nc) as tc:
        def body(_i=None):
            _body(tc, nc, mybir, xb_d, wq_d, wv_d, bq_d, wp_d, gam_d, bet_d,
                  sel_d, selT_d, vones_d, part_d, bv_d, ablate)
        if repeats == 1:
            body()
        elif unroll:
            for _ in range(repeats):
                body()
        else:
            with tc.For_i(0, repeats, 1) as _i:
                body(_i)
    nc.compile()
    return nc


def _body(tc, nc, mybir, xb_d, wq_d, wv_d, bq_d, wp_d, gam_d, bet_d,
          sel_d, selT_d, vones_d, part_d, bv_d, ablate=""):
    from contextlib import ExitStack
    AF = mybir.ActivationFunctionType
    ALU = mybir.AluOpType
    DR = mybir.MatmulPerfMode.DoubleRow
    f32 = mybir.dt.float32
    f32r = mybir.dt.float32r
    f8 = mybir.dt.float8e4
    u8 = mybir.dt.uint8
    ctx = ExitStack()
    with ctx:
        ctx.enter_context(nc.allow_low_precision("fp8/f32r attention"))
        big = ctx.enter_context(tc.tile_pool(name="big", bufs=1))
        xpool = ctx.enter_context(tc.tile_pool(name="x2", bufs=2))
        wpool = ctx.enter_context(tc.tile_pool(name="w", bufs=1))
        small = ctx.enter_context(tc.tile_pool(name="small", bufs=1))
        epool = ctx.enter_context(tc.tile_pool(name="E", bufs=4))
        npool = ctx.enter_context(tc.tile_pool(name="norm", bufs=2))

        # ---------------- load x + weights ----------------
        # small weight tensors ride the gpsimd SWDGE queue (cheap Pool-seq
        # dispatch, doesn't delay the x stream on the SP HWDGE queue); x
        # chunks split across the SP and Pool initiators.
        wq_raw, gam_t, bet_t = [], [], []
        for t in range(2):
            wt = wpool.tile([128, 256], f32, tag=f"wq{t}", name=f"wq{t}")
            nc.gpsimd.dma_start(wt[:], wq_d[t * 128:(t + 1) * 128, :])
            wq_raw.append(wt)
            g = small.tile([128, 1], f32, tag=f"gam{t}", name=f"gam{t}")
            nc.gpsimd.dma_start(g[:], gam_d[t])
            gam_t.append(g)
            bt = small.tile([128, 1], f32, tag=f"bet{t}", name=f"bet{t}")
            nc.gpsimd.dma_start(bt[:], bet_d[t])
            bet_t.append(bt)
        wv_raw = []
        for t in range(2):
            wvt = wpool.tile([128, 128], f32, tag=f"wv{t}", name=f"wv{t}")
            nc.gpsimd.dma_start(wvt[:], wv_d[t * 128:(t + 1) * 128, :])
            wv_raw.append(wvt)
        wp8 = wpool.tile([64, 2, 256], f8, tag="wp8", name="wp8")
        nc.gpsimd.dma_start(wp8[:], wp_d[:])
        bq_t = []
        for blk in range(2):
            bqt = small.tile([128, 1], f32, tag=f"bq{blk}", name=f"bq{blk}")
            nc.gpsimd.dma_start(bqt[:], bq_d[blk])
            bq_t.append(bqt)
        sel = small.tile([128, 4], f32, tag="sel", name="sel")
        nc.gpsimd.dma_start(sel[:], sel_d[:])
        selT = small.tile([4, 128], f32, tag="selT", name="selT")
        nc.gpsimd.dma_start(selT[:], selT_d[:])

        xrt = []
        for t in range(2):
            xtile = xpool.tile([128, HW], f32r, tag=f"xt{t}", name=f"xt{t}")
            for ch in range(4):
                nc.sync.dma_start(xtile[:, ch * 1024:(ch + 1) * 1024],
                                  xb_d[t * 128:(t + 1) * 128,
                                       ch * 1024:(ch + 1) * 1024])
            xrt.append(xtile)
        xr = xrt
        xt = [x.bitcast(f32) for x in xrt]
        eps_t = small.tile([4, 1], f32, tag="eps", name="eps")
        nc.vector.memset(eps_t[:], EPS)
        bias_e = small.tile([128, 1], f32, tag="biasE", name="biasE")
        nc.vector.memset(bias_e[:], BETA_ACT)
        # preload the exp/ln ACT table set while the x DMA streams in
        warm = small.tile([1, 1], f32, tag="warm", name="warm")
        nc.scalar.activation(warm[:], eps_t[0:1, :], AF.Exp)
        nc.scalar.activation(warm[:], warm[:], AF.Ln)

        vT8 = [big.tile([128, 32, 96], f8, tag=f"vT{h}", name=f"vT{h}")
               for h in range(2)]
        for h in range(2):
            nc.sync.dma_start(vT8[h][:, :, 64:66], vones_d[:])

        # ---------------- groupnorm stats ----------------
        stats = []   # per tile [128, 2]: col0 mean_c, col1 E[x^2]_c
        for t in range(2):
            bno = small.tile([128, 8, 6], f32, tag=f"bno{t}", name=f"bno{t}")
            for ch in range(8):
                nc.vector.bn_stats(bno[:, ch, :], xt[t][:, ch * 512:(ch + 1) * 512])
            cst = small.tile([128, 2], f32, tag=f"cst{t}", name=f"cst{t}")
            nc.vector.bn_aggr(cst[:], bno[:])          # (mean_c, var_c)
            st = small.tile([128, 2], f32, tag=f"st{t}", name=f"st{t}")
            nc.vector.tensor_copy(st[:, 0:1], cst[:, 0:1])
            m2c = small.tile([128, 1], f32, tag=f"m2c{t}", name=f"m2c{t}")
            nc.vector.tensor_tensor(m2c[:], cst[:, 0:1], cst[:, 0:1], op=ALU.mult)
            nc.vector.tensor_tensor(st[:, 1:2], cst[:, 1:2], m2c[:], op=ALU.add)
            stats.append(st)

        with tc.tile_pool(name="ps_gn", bufs=1, space="PSUM") as ps_gn:
            psg = ps_gn.tile([4, 4], f32, tag="psg", name="psg")
            for t in range(2):
                nc.tensor.matmul(psg[:, 2 * t:2 * t + 2], sel[:], stats[t][:],
                                 start=True, stop=True)
            gmr = []   # per tile [4, 2]: col0 mean_g, col1 rstd_g
            for t in range(2):
                gm = small.tile([4, 2], f32, tag=f"gmr{t}", name=f"gmr{t}")
                nc.vector.tensor_scalar_mul(gm[:, 0:1], psg[:, 2 * t:2 * t + 1],
                                            1.0 / 32.0)
                m2 = small.tile([4, 1], f32, tag=f"m2{t}", name=f"m2{t}")
                nc.vector.tensor_tensor(m2[:], gm[:, 0:1], gm[:, 0:1], op=ALU.mult)
                var = small.tile([4, 1], f32, tag=f"var{t}", name=f"var{t}")
                nc.vector.scalar_tensor_tensor(var[:], psg[:, 2 * t + 1:2 * t + 2],
                                               1.0 / 32.0, m2[:],
                                               op0=ALU.mult, op1=ALU.subtract)
                lnv = small.tile([4, 1], f32, tag=f"lnv{t}", name=f"lnv{t}")
                nc.scalar.activation(lnv[:], var[:], AF.Ln, bias=eps_t[:])
                nc.scalar.activation(gm[:, 1:2], lnv[:], AF.Exp, scale=-0.5)
                gmr.append(gm)

            # per-channel scale/shift; fold into weights
            w_s, wv_s, t_r = [], [], []
            for t in range(2):
                psc = ps_gn.tile([128, 2], f32, tag="psc", name="psc")
                nc.tensor.matmul(psc[:], selT[:], gmr[t][:], start=True, stop=True)
                s_t = small.tile([128, 1], f32, tag=f"s{t}", name=f"s{t}")
                nc.vector.tensor_tensor(s_t[:], psc[:, 1:2], gam_t[t][:], op=ALU.mult)
                ms = small.tile([128, 1], f32, tag=f"ms{t}", name=f"ms{t}")
                nc.vector.tensor_tensor(ms[:], psc[:, 0:1], s_t[:], op=ALU.mult)
                tr = small.tile([128, 1], f32, tag=f"t{t}", name=f"t{t}")
                nc.vector.tensor_tensor(tr[:], bet_t[t][:], ms[:], op=ALU.subtract)
                t_r.append(tr)
                ws = wpool.tile([128, 256], f32r, tag=f"ws{t}", name=f"ws{t}")
                nc.vector.tensor_scalar_mul(ws[:], wq_raw[t][:], s_t[:])
                w_s.append(ws)
                wvs = wpool.tile([128, 128], f32r, tag=f"wvs{t}", name=f"wvs{t}")
                nc.vector.tensor_scalar_mul(wvs[:], wv_raw[t][:], s_t[:])
                wv_s.append(wvs)

            # q bias fold: b'[o] = bq[o] + sum_c Wq[o,c] * t_c   (Wq x log2e)
            # v bias:      bv[o] = bqv[o] + sum_c Wv[o,c] * t_c  -> host
            psb = ps_gn.tile([128, 2], f32, tag="psb", name="psb")
            for t in range(2):
                nc.tensor.matmul(psb[:, 0:1], wq_raw[t][:, 0:128], t_r[t][:],
                                 start=(t == 0), stop=(t == 1))
            for t in range(2):
                nc.tensor.matmul(psb[:, 1:2], wv_raw[t][:], t_r[t][:],
                                 start=(t == 0), stop=(t == 1))
            bias_q = small.tile([128, 1], f32, tag="biasq", name="bias_q")
            nc.vector.tensor_tensor(bias_q[:], psb[:, 0:1], bq_t[0][:], op=ALU.add)
            bv_sb = small.tile([128, 1], f32, tag="bvsb", name="bv_sb")
            nc.scalar.activation(bv_sb[:], psb[:, 1:2], AF.Identity,
                                 bias=bq_t[1][:])
            nc.sync.dma_start(bv_d[:], bv_sb[:])

        # ---------------- K/Q GEMM (f32r) ----------------
        k_sb = big.tile([128, HW], f32r, tag="ksb", name="ksb")
        q_sb = big.tile([128, HW], f32r, tag="qsb", name="qsb")
        with tc.tile_pool(name="ps_kq", bufs=2, space="PSUM") as ps_kq, \
             tc.tile_pool(name="ps_v", bufs=2, space="PSUM") as ps_v:
            for blk, dst in ((1, k_sb), (0, q_sb)):       # K first
                for g in range(4):
                    ps = ps_kq.tile([128, 2, 512], f32, tag="pskq", name="pskq")
                    for j in range(2):
                        nsl = slice((2 * g + j) * 512, (2 * g + j + 1) * 512)
                        for t in range(2):
                            nc.tensor.matmul(
                                ps[:, j, :],
                                w_s[t][:, blk * 128:(blk + 1) * 128],
                                xr[t][:, nsl], start=(t == 0), stop=(t == 1))
                    gsl = slice(g * 1024, (g + 1) * 1024)
                    if blk == 1:      # K: plain copy (bias cancels in softmax)
                        if g % 2 == 0:
                            nc.scalar.activation(dst[:, gsl], ps[:], AF.Copy)
                        else:
                            nc.vector.tensor_copy(dst[:, gsl], ps[:])
                    else:             # Q: add folded bias
                        if g % 2 == 0:
                            nc.scalar.activation(dst[:, gsl], ps[:], AF.Identity,
                                                 bias=bias_q[:])
                        else:
                            nc.vector.tensor_scalar(dst[:, gsl], ps[:],
                                                    bias_q[:], None, op0=ALU.add)

            # ------- V GEMM, pre-transposed: out[pix, (h,hd)] ----------
            for grp in range(8):
                psV = ps_v.tile([128, 4, 128], f32, tag="psv", name="psv")
                for c4 in range(4):
                    chunk = grp * 4 + c4
                    csl = slice(chunk * 128, (chunk + 1) * 128)
                    for t in range(2):
                        nc.tensor.matmul(psV[:, c4, :], xr[t][:, csl],
                                         wv_s[t][:], start=(t == 0), stop=(t == 1))
                for h in range(2):
                    src = psV[:, :, h * 64:(h + 1) * 64]
                    dst = vT8[h][:, grp * 4:(grp + 1) * 4, 0:64]
                    if (grp + h) % 2 == 0:
                        nc.scalar.activation(dst, src, AF.Copy)
                    else:
                        nc.vector.tensor_copy(dst, src)

        # ---------------- attention ----------------
        attn8 = big.tile([64, 2, HW], f8, tag="attn8", name="attn8")
        cA = EXP_SPLIT
        with tc.tile_pool(name="ps_s", bufs=3, space="PSUM") as ps_sp, \
             tc.tile_pool(name="ps_o", bufs=1, space="PSUM") as ps_op, \
             tc.tile_pool(name="prout", bufs=2) as prout:
            def proj(qi):
                # output projection (fp8 DR); deferred into the next qi's
                # stream so the PE FIFO never waits on the gpsimd normalize.
                # psP borrows slots from the S-staging ring (PSUM is 8 banks:
                # 3x2 staging + 2 ps_o).
                qsl = slice(qi * 512, (qi + 1) * 512)
                for mch in range(2):
                    psP = ps_sp.tile([128, 512], f32, tag="pss", name="psP")
                    nc.tensor.matmul(psP[:], wp8[:, :, mch * 128:(mch + 1) * 128],
                                     attn8[:, :, qsl], start=True, stop=True,
                                     perf_mode=DR)
                    osb = prout.tile([128, 512], f32, tag="posb", name="posb")
                    if (qi + mch) % 2 == 0:
                        nc.scalar.activation(osb[:], psP[:], AF.Copy)
                    else:
                        nc.vector.tensor_copy(osb[:], psP[:])
                    nc.sync.dma_start(part_d[mch * 128:(mch + 1) * 128, qsl],
                                      osb[:])

            def normalize(qi, ocp_t, rcp_t):
                # gpsimd broadcast+mult (deferred: ~4us of Pool latency that
                # must ride under the next qi's exp stream).  For the last qi
                # the mults go on the then-idle DVE to shorten the tail.
                qsl = slice(qi * 512, (qi + 1) * 512)
                bcs = []
                for h in range(2):
                    bc = npool.tile([64, 512], f32r, tag="bc", name="bc")
                    nc.gpsimd.partition_broadcast(bc[:], rcp_t[h][:], channels=64)
                    bcs.append(bc)
                    if qi < 7:
                        nc.gpsimd.tensor_tensor(attn8[:, h, qsl], ocp_t[h][:],
                                                bc[:], op=ALU.mult)
                if qi == 7:
                    for h in range(2):
                        nc.vector.tensor_tensor(attn8[:, h, qsl], ocp_t[h][:],
                                                bcs[h][:], op=ALU.mult)

            pending = None   # (qi, ocp_tiles, rcp_tiles) awaiting normalize+proj
            for qi in range(8):
                qsl = slice(qi * 512, (qi + 1) * 512)
                ps_o = [ps_op.tile([66, 512], f32, tag=f"pso{h}", name=f"pso{h}")
                        for h in range(2)]

                def mm1_exp(c, E2):
                    # both heads concurrently: h0 in PE rows 0-63, h1 in 64-127
                    ps = ps_sp.tile([128, 2, 512], f32, tag="pss", name="pss")
                    csl = slice(c * 128, (c + 1) * 128)
                    for h in range(2):
                        hsl = slice(h * 64, (h + 1) * 64)
                        nc.tensor.matmul(ps[:, h, :], k_sb[hsl, csl],
                                         q_sb[hsl, qsl], start=True, stop=True)
                    # Bresenham-interleaved ACT/DVE split (cA of 32 on ACT)
                    if (c + 1) * cA // 32 > c * cA // 32:
                        nc.scalar.activation(E2[:, c & 1, :, :], ps[:], AF.Exp,
                                             scale=LN2 / 8.0, bias=bias_e[:])
                    else:
                        nc.vector.tensor_scalar(E2.bitcast(u8)[:, c & 1, :, :],
                                                ps[:], B_SCH, None, op0=ALU.add)

                def mm2(cp, E2):
                    for h in range(2):
                        nc.tensor.matmul(ps_o[h][:],
                                         vT8[h][:, 2 * cp:2 * cp + 2, 0:66],
                                         E2[:, :, h, :], start=(cp == 0),
                                         stop=(cp == 15), perf_mode=DR)

                Es = []
                for cp in range(16):
                    E2 = epool.tile([128, 2, 2, 512], f8, tag="E2", name="E2")
                    Es.append(E2)
                    mm1_exp(2 * cp, E2)
                    mm1_exp(2 * cp + 1, E2)
                    if cp >= 2:        # lag 2 so the qi-boundary mm2 never
                        mm2(cp - 2, Es[cp - 2])   # heads the PE FIFO early
                    if pending is not None and cp == 2:
                        normalize(*pending)
                    if pending is not None and cp == 6:
                        proj(pending[0])
                        pending = None
                mm2(14, Es[14])
                mm2(15, Es[15])

                # immediate epilogue: numerator + reciprocal off PSUM fast so
                # the single-buffered ps_o frees for the next qi's mm2
                ocp_t, rcp_t = [], []
                for h in range(2):
                    ocp = npool.tile([64, 512], f32r, tag="ocp", name="ocp")
                    nc.scalar.activation(ocp[:], ps_o[h][0:64, :], AF.Copy)
                    rcp = npool.tile([1, 512], f32r, tag="rcp", name="rcp")
                    nc.vector.reciprocal(rcp[:], ps_o[h][64:65, :])
                    ocp_t.append(ocp)
                    rcp_t.append(rcp)
                pending = (qi, ocp_t, rcp_t)
            normalize(*pending)
            proj(7)


def _get_nc(repeats=1, ablate="", unroll=False):
    key = (repeats, ablate, unroll)
    if key not in _CACHE:
        _CACHE[key] = _build(repeats, ablate, unroll)
    return _CACHE[key]


def make_in_maps(x, gamma, beta, w_qkv, b_qkv, w_proj, b_proj):
    x = np.asarray(x, dtype=np.float32)
    gamma = np.asarray(gamma, dtype=np.float32)
    beta = np.asarray(beta, dtype=np.float32)
    w_qkv = np.asarray(w_qkv, dtype=np.float32)
    b_qkv = np.asarray(b_qkv, dtype=np.float32)
    w_proj = np.asarray(w_proj, dtype=np.float32)

    gam_in = np.ascontiguousarray(gamma.reshape(2, 128, 1))
    bet_in = np.ascontiguousarray(beta.reshape(2, 128, 1))
    sel_in = np.zeros((128, 4), dtype=np.float32)
    for g in range(4):
        sel_in[g * 32:(g + 1) * 32, g] = 1.0
    selT_in = np.ascontiguousarray(sel_in.T)
    vones_in = np.zeros((128, 32, 2), dtype=ml_dtypes.float8_e4m3)
    vones_in[:, :, 0] = 1.0 / VSCALE
    in_maps = []
    for core in range(NCORES):
        b, hp = core // 2, core % 2
        rs = slice(hp * 128, (hp + 1) * 128)
        wq_s = np.concatenate([w_qkv[rs] * LOG2E,
                               w_qkv[256:][rs.start:rs.stop]], axis=0)  # [256, 256]
        wv_s = w_qkv[512:][rs.start:rs.stop]                            # [128, 256]
        # wp8[r, h, o] = w_proj[o, hp*128 + h*64 + r] * WSCALE
        wp_slice = w_proj[:, rs].T.reshape(2, 64, 256)          # [h, r, o]
        wp8 = np.ascontiguousarray(
            wp_slice.transpose(1, 0, 2) * WSCALE).astype(ml_dtypes.float8_e4m3)
        in_maps.append({
            "xb": np.ascontiguousarray(x[b].reshape(256, HW)),
            "wq": np.ascontiguousarray(wq_s.T),
            "wv": np.ascontiguousarray(wv_s.T),
            "bq": np.ascontiguousarray(
                np.stack([b_qkv[rs] * LOG2E,
                          b_qkv[512 + rs.start:512 + rs.stop]])[:, :, None]),
            "wp8": wp8,
            "gam": gam_in,
            "bet": bet_in,
            "selc": sel_in,
            "selT": selT_in,
            "vones": vones_in,
        })
    return in_maps


def assemble(x, w_proj, b_proj, results):
    w_proj = np.asarray(w_proj, dtype=np.float32)
    out = np.empty((B, C, H, W), dtype=np.float32)
    scale = 1.0 / (VSCALE * WSCALE)
    for b in range(B):
        acc = (results[2 * b]["part"] + results[2 * b + 1]["part"]) * scale
        for hp in range(2):
            rs = slice(hp * 128, (hp + 1) * 128)
            bv = results[2 * b + hp]["bv"][:, 0]
            acc += (w_proj[:, rs] @ bv)[:, None]
        acc += np.asarray(b_proj, dtype=np.float32)[:, None]
        out[b] = (np.asarray(x[b], dtype=np.float32).reshape(C, HW) + acc
                  ).reshape(C, H, W)
    return out


def kernel(x, gamma, beta, w_qkv, b_qkv, w_proj, b_proj):
    from concourse.bass_utils import run_bass_kernel_spmd
    nc = _get_nc()
    in_maps = make_in_maps(x, gamma, beta, w_qkv, b_qkv, w_proj, b_proj)
    res = run_bass_kernel_spmd(nc, in_maps, core_ids=list(range(NCORES)))
    return assemble(x, w_proj, b_proj, res.results)


# revision 33
# speedup vs baseline: 1.1826x; 1.1826x over previous
"""AttnBlock (GroupNorm + 4-head hd-64 self-attention + proj + residual)
Trainium2 Bass kernel, 8 NeuronCores.

Sharding: core i handles batch b = i//2 and head-pair hp = i%2 (heads 2hp, 2hp+1).
Each core computes GroupNorm stats for its batch (folded into the QKV GEMMs as a
per-channel affine on the weights/bias), runs flash-style attention for its two
heads on-chip, and emits partial[o, pix] = sum_{c in its 128 ch} w_proj[o,c]*attn.
Host: out[b] = x[b] + b_proj + sum_hp(partial[hp]/128 + w_proj[:,hp]@bv[hp]).

Structure (the kernel is ACT/DVE-bound: every S element must exit PSUM through
one of the two engines that can read PSUM):
- mm1 (QK^T, f32r) is ROW-TILED: head0 occupies PE rows 0-63, head1 rows 64-127
  (contraction is only hd=64), so both heads' matmuls run CONCURRENTLY in the
  array -> one [128kpix, 2(head), 512q] PSUM tile per 216ns window, and the PE
  cost of mm1 halves vs zero-padded k. No kz padding tiles needed.
- V is produced PRE-TRANSPOSED by the QKV GEMM itself: lhsT = x-chunk
  (stationary), rhs = w_v^T -> out[pix, (h,hd)] accumulates in PSUM; one fp8
  exit per head per 4-chunk bank writes vT8 directly. No PE-transpose pass,
  no v_sb, half the exits.
- Softmax exp splits across ACT (true exp via table) and DVE (Schraudolph:
  round(S'+24) bit pattern as fp8e4m3, computing 2^((S'-32)/8)); log2e is
  pre-folded into the Q weights on the host so both paths are 1 op.
- Biases: K-bias dropped entirely (adds a per-query constant to S -> softmax
  invariant). V-bias returned to host (softmax weights sum to 1, so it adds
  W_proj@bv to the output). Q-bias folded into the Q-exit epilogue.
- mm2 (attn@V) and proj run fp8 DoubleRow; denominator comes out of mm2 via an
  extra 1/32 ones-column in vT8; normalize (reciprocal+broadcast+mult) is on
  DVE/gpsimd off the exit-engine critical path; proj is pipelined per-qi.
"""

import numpy as np
import ml_dtypes

B, C, H, W = 4, 256, 64, 64
HW = H * W            # 4096 pixels
NH = 4                # heads
HD = 64               # head dim
NG = 8                # groupnorm groups
EPS = 1e-5
NCORES = 8

LOG2E = 1.4426950408889634
LN2 = 0.6931471805599453
B_SCH = 24.0                      # schraudolph bias: bits = round(S' + B)
# S' = log2e*S_raw (log2e folded into Q weights).  max raw S = 62.7 ->
# S' = 90.5 -> max bits 114 < 120 (fp8e4 inf); low tail clamps to 0.
BETA_ACT = (B_SCH - 56.0) / 8.0 * LN2   # ACT path: exp(S'*ln2/8 + beta)
VSCALE = 32.0                     # denominator ones col = 1/32 -> attn x32
WSCALE = 4.0                      # w_proj stored x4
# exp engine split: per 32 chunk-blocks of a qi, how many go to ACT (rest DVE).
EXP_SPLIT = 17

_CACHE = {}


def _build(repeats=1, ablate="", unroll=False):
    import concourse.tile as tile
    from concourse import bacc, mybir

    f32 = mybir.dt.float32
    f8 = mybir.dt.float8e4

    nc = bacc.Bacc("TRN2", target_bir_lowering=False, debug=False,
                   enable_asserts=False, num_devices=NCORES)

    xb_d = nc.dram_tensor("xb", [256, HW], mybir.dt.float32r,
                          kind="ExternalInput").ap()
    wq_d = nc.dram_tensor("wq", [256, 256], f32, kind="ExternalInput").ap()   # [c, o] lhsT; o = q|k blocks of 128 (q cols pre-scaled by log2e)
    wv_d = nc.dram_tensor("wv", [256, 128], f32, kind="ExternalInput").ap()   # [c, (h,hd)] rhs for transposed V GEMM
    bq_d = nc.dram_tensor("bq", [2, 128, 1], f32, kind="ExternalInput").ap()  # [0]=q bias (x log2e), [1]=v bias
    wp_d = nc.dram_tensor("wp8", [64, 2, 256], f8, kind="ExternalInput").ap() # [r, h, o] x4
    gam_d = nc.dram_tensor("gam", [2, 128, 1], f32, kind="ExternalInput").ap()
    bet_d = nc.dram_tensor("bet", [2, 128, 1], f32, kind="ExternalInput").ap()
    sel_d = nc.dram_tensor("selc", [128, 4], f32, kind="ExternalInput").ap()
    selT_d = nc.dram_tensor("selT", [4, 128], f32, kind="ExternalInput").ap()
    vones_d = nc.dram_tensor("vones", [128, 32, 2], f8, kind="ExternalInput").ap()
    idq_d = nc.dram_tensor("idq", [128, 64], mybir.dt.float32r,
                           kind="ExternalInput").ap()
    part_d = nc.dram_tensor("part", [256, HW], f32, kind="ExternalOutput").ap()
    bv_d = nc.dram_tensor("bv", [128, 1], f32, kind="ExternalOutput").ap()

    with tile.TileContext(nc) as tc:
        def body(_i=None):
            _body(tc, nc, mybir, xb_d, wq_d, wv_d, bq_d, wp_d, gam_d, bet_d,
                  sel_d, selT_d, vones_d, idq_d, part_d, bv_d, ablate)
        if repeats == 1:
            body()
        elif unroll:
            for _ in range(repeats):
                body()
        else:
            with tc.For_i(0, repeats, 1) as _i:
                body(_i)
    nc.compile()
    return nc


def _body(tc, nc, mybir, xb_d, wq_d, wv_d, bq_d, wp_d, gam_d, bet_d,
          sel_d, selT_d, vones_d, idq_d, part_d, bv_d, ablate=""):
    from contextlib import ExitStack
    AF = mybir.ActivationFunctionType
    ALU = mybir.AluOpType
    DR = mybir.MatmulPerfMode.DoubleRow
    f32 = mybir.dt.float32
    f32r = mybir.dt.float32r
    f8 = mybir.dt.float8e4
    u8 = mybir.dt.uint8
    ctx = ExitStack()
    with ctx:
        ctx.enter_context(nc.allow_low_precision("fp8/f32r attention"))
        big = ctx.enter_context(tc.tile_pool(name="big", bufs=1))
        xpool = ctx.enter_context(tc.tile_pool(name="x2", bufs=2))
        wpool = ctx.enter_context(tc.tile_pool(name="w", bufs=1))
        small = ctx.enter_context(tc.tile_pool(name="small", bufs=1))
        epool = ctx.enter_context(tc.tile_pool(name="E", bufs=4))
        npool = ctx.enter_context(tc.tile_pool(name="norm", bufs=2))

        # ---------------- load x + weights ----------------
        # small weight tensors ride the gpsimd SWDGE queue (cheap Pool-seq
        # dispatch, doesn't delay the x stream on the SP HWDGE queue); x
        # chunks split across the SP and Pool initiators.
        sdma = nc.sync if "spdma" in ablate else nc.gpsimd
        wq_raw, gam_t, bet_t = [], [], []
        for t in range(2):
            wt = wpool.tile([128, 256], f32, tag=f"wq{t}", name=f"wq{t}")
            sdma.dma_start(wt[:], wq_d[t * 128:(t + 1) * 128, :])
            wq_raw.append(wt)
            g = small.tile([128, 1], f32, tag=f"gam{t}", name=f"gam{t}")
            sdma.dma_start(g[:], gam_d[t])
            gam_t.append(g)
            bt = small.tile([128, 1], f32, tag=f"bet{t}", name=f"bet{t}")
            sdma.dma_start(bt[:], bet_d[t])
            bet_t.append(bt)
        wv_raw = []
        for t in range(2):
            wvt = wpool.tile([128, 128], f32, tag=f"wv{t}", name=f"wv{t}")
            sdma.dma_start(wvt[:], wv_d[t * 128:(t + 1) * 128, :])
            wv_raw.append(wvt)
        wp8 = wpool.tile([64, 2, 256], f8, tag="wp8", name="wp8")
        sdma.dma_start(wp8[:], wp_d[:])
        bq_t = []
        for blk in range(2):
            bqt = small.tile([128, 1], f32, tag=f"bq{blk}", name=f"bq{blk}")
            sdma.dma_start(bqt[:], bq_d[blk])
            bq_t.append(bqt)
        sel = small.tile([128, 4], f32, tag="sel", name="sel")
        sdma.dma_start(sel[:], sel_d[:])
        selT = small.tile([4, 128], f32, tag="selT", name="selT")
        sdma.dma_start(selT[:], selT_d[:])

        xrt = []
        for t in range(2):
            xtile = xpool.tile([128, HW], f32r, tag=f"xt{t}", name=f"xt{t}")
            for ch in range(4):
                nc.sync.dma_start(xtile[:, ch * 1024:(ch + 1) * 1024],
                                  xb_d[t * 128:(t + 1) * 128,
                                       ch * 1024:(ch + 1) * 1024])
            xrt.append(xtile)
        xr = xrt
        xt = [x.bitcast(f32) for x in xrt]
        eps_t = small.tile([4, 1], f32, tag="eps", name="eps")
        nc.vector.memset(eps_t[:], EPS)
        bias_e = small.tile([128, 1], f32, tag="biasE", name="biasE")
        nc.vector.memset(bias_e[:], BETA_ACT)
        # preload the exp/ln ACT table set while the x DMA streams in
        warm = small.tile([1, 1], f32, tag="warm", name="warm")
        nc.scalar.activation(warm[:], eps_t[0:1, :], AF.Exp)
        nc.scalar.activation(warm[:], warm[:], AF.Ln)

        vT8 = [big.tile([128, 32, 96], f8, tag=f"vT{h}", name=f"vT{h}")
               for h in range(2)]
        for h in range(2):
            nc.sync.dma_start(vT8[h][:, :, 64:66], vones_d[:])

        # ---------------- groupnorm stats ----------------
        stats = []   # per tile [128, 2]: col0 mean_c, col1 E[x^2]_c
        for t in range(2):
            bno = small.tile([128, 8, 6], f32, tag=f"bno{t}", name=f"bno{t}")
            for ch in range(8):
                nc.vector.bn_stats(bno[:, ch, :], xt[t][:, ch * 512:(ch + 1) * 512])
            cst = small.tile([128, 2], f32, tag=f"cst{t}", name=f"cst{t}")
            nc.vector.bn_aggr(cst[:], bno[:])          # (mean_c, var_c)
            st = small.tile([128, 2], f32, tag=f"st{t}", name=f"st{t}")
            nc.vector.tensor_copy(st[:, 0:1], cst[:, 0:1])
            m2c = small.tile([128, 1], f32, tag=f"m2c{t}", name=f"m2c{t}")
            nc.vector.tensor_tensor(m2c[:], cst[:, 0:1], cst[:, 0:1], op=ALU.mult)
            nc.vector.tensor_tensor(st[:, 1:2], cst[:, 1:2], m2c[:], op=ALU.add)
            stats.append(st)

        with tc.tile_pool(name="ps_gn", bufs=1, space="PSUM") as ps_gn:
            psg = ps_gn.tile([4, 4], f32, tag="psg", name="psg")
            for t in range(2):
                nc.tensor.matmul(psg[:, 2 * t:2 * t + 2], sel[:], stats[t][:],
                                 start=True, stop=True)
            gmr = []   # per tile [4, 2]: col0 mean_g, col1 rstd_g
            for t in range(2):
                gm = small.tile([4, 2], f32, tag=f"gmr{t}", name=f"gmr{t}")
                nc.vector.tensor_scalar_mul(gm[:, 0:1], psg[:, 2 * t:2 * t + 1],
                                            1.0 / 32.0)
                m2 = small.tile([4, 1], f32, tag=f"m2{t}", name=f"m2{t}")
                nc.vector.tensor_tensor(m2[:], gm[:, 0:1], gm[:, 0:1], op=ALU.mult)
                var = small.tile([4, 1], f32, tag=f"var{t}", name=f"var{t}")
                nc.vector.scalar_tensor_tensor(var[:], psg[:, 2 * t + 1:2 * t + 2],
                                               1.0 / 32.0, m2[:],
                                               op0=ALU.mult, op1=ALU.subtract)
                lnv = small.tile([4, 1], f32, tag=f"lnv{t}", name=f"lnv{t}")
                nc.scalar.activation(lnv[:], var[:], AF.Ln, bias=eps_t[:])
                nc.scalar.activation(gm[:, 1:2], lnv[:], AF.Exp, scale=-0.5)
                gmr.append(gm)

            # per-channel scale/shift; fold into weights
            w_s, wv_s, t_r = [], [], []
            for t in range(2):
                psc = ps_gn.tile([128, 2], f32, tag="psc", name="psc")
                nc.tensor.matmul(psc[:], selT[:], gmr[t][:], start=True, stop=True)
                s_t = small.tile([128, 1], f32, tag=f"s{t}", name=f"s{t}")
                nc.vector.tensor_tensor(s_t[:], psc[:, 1:2], gam_t[t][:], op=ALU.mult)
                ms = small.tile([128, 1], f32, tag=f"ms{t}", name=f"ms{t}")
                nc.vector.tensor_tensor(ms[:], psc[:, 0:1], s_t[:], op=ALU.mult)
                tr = small.tile([128, 1], f32, tag=f"t{t}", name=f"t{t}")
                nc.vector.tensor_tensor(tr[:], bet_t[t][:], ms[:], op=ALU.subtract)
                t_r.append(tr)
                ws = wpool.tile([128, 256], f32r, tag=f"ws{t}", name=f"ws{t}")
                nc.vector.tensor_scalar_mul(ws[:], wq_raw[t][:], s_t[:])
                w_s.append(ws)
                wvs = wpool.tile([128, 128], f32r, tag=f"wvs{t}", name=f"wvs{t}")
                nc.vector.tensor_scalar_mul(wvs[:], wv_raw[t][:], s_t[:])
                wv_s.append(wvs)

            # q bias fold: b'[o] = bq[o] + sum_c Wq[o,c] * t_c   (Wq x log2e)
            # v bias:      bv[o] = bqv[o] + sum_c Wv[o,c] * t_c  -> host
            psb = ps_gn.tile([128, 2], f32, tag="psb", name="psb")
            for t in range(2):
                nc.tensor.matmul(psb[:, 0:1], wq_raw[t][:, 0:128], t_r[t][:],
                                 start=(t == 0), stop=(t == 1))
            for t in range(2):
                nc.tensor.matmul(psb[:, 1:2], wv_raw[t][:], t_r[t][:],
                                 start=(t == 0), stop=(t == 1))
            bias_q = small.tile([128, 1], f32, tag="biasq", name="bias_q")
            nc.vector.tensor_tensor(bias_q[:], psb[:, 0:1], bq_t[0][:], op=ALU.add)
            bv_sb = small.tile([128, 1], f32, tag="bvsb", name="bv_sb")
            nc.scalar.activation(bv_sb[:], psb[:, 1:2], AF.Identity,
                                 bias=bq_t[1][:])
            nc.sync.dma_start(bv_d[:], bv_sb[:])

        # ---------------- K/Q GEMM (f32r) ----------------
        kzpad = "kzpad" in ablate
        q_sb = big.tile([128, HW], f32r, tag="qsb", name="qsb")
        if kzpad:
            kz = [big.tile([128, HW], f32r, tag=f"kz{h}", name=f"kz{h}")
                  for h in range(2)]
            nc.gpsimd.memset(kz[0][64:128, :].bitcast(f32), 0.0)
            nc.gpsimd.memset(kz[1][0:64, :].bitcast(f32), 0.0)
            k_sb = None
        else:
            k_sb = big.tile([128, HW], f32r, tag="ksb", name="ksb")
        with tc.tile_pool(name="ps_kq", bufs=2, space="PSUM") as ps_kq, \
             tc.tile_pool(name="ps_v", bufs=2, space="PSUM") as ps_v:
            for blk, dst in ((1, k_sb), (0, q_sb)):       # K first
                for g in range(4):
                    ps = ps_kq.tile([128, 2, 512], f32, tag="pskq", name="pskq")
                    for j in range(2):
                        nsl = slice((2 * g + j) * 512, (2 * g + j + 1) * 512)
                        for t in range(2):
                            nc.tensor.matmul(
                                ps[:, j, :],
                                w_s[t][:, blk * 128:(blk + 1) * 128],
                                xr[t][:, nsl], start=(t == 0), stop=(t == 1))
                    gsl = slice(g * 1024, (g + 1) * 1024)
                    if blk == 1 and kzpad:   # K into zero-padded per-head tiles
                        for h in range(2):
                            hs = slice(h * 64, (h + 1) * 64)
                            kd = kz[h][hs, gsl]
                            if (g + h) % 2 == 0:
                                nc.scalar.activation(kd, ps[hs, :, :], AF.Copy)
                            else:
                                nc.vector.tensor_copy(kd, ps[hs, :, :])
                    elif blk == 1:    # K: plain copy (bias cancels in softmax)
                        if g % 2 == 0:
                            nc.scalar.activation(dst[:, gsl], ps[:], AF.Copy)
                        else:
                            nc.vector.tensor_copy(dst[:, gsl], ps[:])
                    else:             # Q: add folded bias
                        if g % 2 == 0:
                            nc.scalar.activation(dst[:, gsl], ps[:], AF.Identity,
                                                 bias=bias_q[:])
                        else:
                            nc.vector.tensor_scalar(dst[:, gsl], ps[:],
                                                    bias_q[:], None, op0=ALU.add)

            if "vold" in ablate:
                # baseline path: plain V GEMM then PE transposes
                idq = small.tile([128, 64], mybir.dt.float32r, tag="idq",
                                 name="idq")
                sdma.dma_start(idq[:], idq_d[:])
                v_sb = big.tile([128, HW], f32r, tag="vsb", name="vsb")
                for g in range(4):
                    ps = ps_kq.tile([128, 2, 512], f32, tag="pskq", name="pskq")
                    for j in range(2):
                        nsl = slice((2 * g + j) * 512, (2 * g + j + 1) * 512)
                        for t in range(2):
                            nc.tensor.matmul(
                                ps[:, j, :], wv_s[t][:],
                                xr[t][:, nsl], start=(t == 0), stop=(t == 1))
                    gsl = slice(g * 1024, (g + 1) * 1024)
                    if g % 2 == 0:
                        nc.scalar.activation(v_sb[:, gsl], ps[:], AF.Copy)
                    else:
                        nc.vector.tensor_copy(v_sb[:, gsl], ps[:])
                for h in range(2):
                    for grp in range(4):
                        pst = ps_v.tile([128, 512], f32r, tag="psv2", name="psv2")
                        for j in range(8):
                            chunk = grp * 8 + j
                            nc.tensor.transpose(
                                pst[:, j * 64:(j + 1) * 64],
                                v_sb[h * 64:(h + 1) * 64,
                                     chunk * 128:(chunk + 1) * 128],
                                idq[h * 64:(h + 1) * 64, 0:64])
                        if (grp + h) % 2 == 0:
                            nc.scalar.activation(
                                vT8[h][:, grp * 8:(grp + 1) * 8, 0:64],
                                pst[:].rearrange("p (j d) -> p j d", d=64),
                                AF.Copy)
                        else:
                            nc.vector.tensor_copy(
                                vT8[h][:, grp * 8:(grp + 1) * 8, 0:64],
                                pst[:].rearrange("p (j d) -> p j d", d=64))
            else:
                # ------- V GEMM, pre-transposed: out[pix, (h,hd)] ----------
                for grp in range(8):
                    psV = ps_v.tile([128, 4, 128], f32, tag="psv", name="psv")
                    for c4 in range(4):
                        chunk = grp * 4 + c4
                        csl = slice(chunk * 128, (chunk + 1) * 128)
                        for t in range(2):
                            nc.tensor.matmul(psV[:, c4, :], xr[t][:, csl],
                                             wv_s[t][:], start=(t == 0),
                                             stop=(t == 1))
                    for h in range(2):
                        src = psV[:, :, h * 64:(h + 1) * 64]
                        dst = vT8[h][:, grp * 4:(grp + 1) * 4, 0:64]
                        if (grp + h) % 2 == 0:
                            nc.scalar.activation(dst, src, AF.Copy)
                        else:
                            nc.vector.tensor_copy(dst, src)

        # ---------------- attention ----------------
        attn8 = big.tile([64, 2, HW], f8, tag="attn8", name="attn8")
        cA = EXP_SPLIT
        import re as _re
        m = _re.search(r"cs(\d+)", ablate)
        if m:
            cA = int(m.group(1))
        with tc.tile_pool(name="ps_s", bufs=3, space="PSUM") as ps_sp, \
             tc.tile_pool(name="ps_o", bufs=1, space="PSUM") as ps_op, \
             tc.tile_pool(name="prout", bufs=2) as prout:
            def proj(qi):
                # output projection (fp8 DR); deferred into the next qi's
                # stream so the PE FIFO never waits on the gpsimd normalize.
                # psP borrows slots from the S-staging ring (PSUM is 8 banks:
                # 3x2 staging + 2 ps_o).
                qsl = slice(qi * 512, (qi + 1) * 512)
                for mch in range(2):
                    psP = ps_sp.tile([128, 512], f32, tag="pss", name="psP")
                    nc.tensor.matmul(psP[:], wp8[:, :, mch * 128:(mch + 1) * 128],
                                     attn8[:, :, qsl], start=True, stop=True,
                                     perf_mode=DR)
                    osb = prout.tile([128, 512], f32, tag="posb", name="posb")
                    if (qi + mch) % 2 == 0:
                        nc.scalar.activation(osb[:], psP[:], AF.Copy)
                    else:
                        nc.vector.tensor_copy(osb[:], psP[:])
                    nc.sync.dma_start(part_d[mch * 128:(mch + 1) * 128, qsl],
                                      osb[:])

            def normalize(qi, ocp_t, rcp_t):
                # gpsimd broadcast+mult (deferred: ~4us of Pool latency that
                # must ride under the next qi's exp stream).  For the last qi
                # the mults go on the then-idle DVE to shorten the tail.
                qsl = slice(qi * 512, (qi + 1) * 512)
                bcs = []
                for h in range(2):
                    bc = npool.tile([64, 512], f32r, tag="bc", name="bc")
                    nc.gpsimd.partition_broadcast(bc[:], rcp_t[h][:], channels=64)
                    bcs.append(bc)
                    if qi < 7:
                        nc.gpsimd.tensor_tensor(attn8[:, h, qsl], ocp_t[h][:],
                                                bc[:], op=ALU.mult)
                if qi == 7:
                    for h in range(2):
                        nc.vector.tensor_tensor(attn8[:, h, qsl], ocp_t[h][:],
                                                bcs[h][:], op=ALU.mult)

            pending = None   # (qi, ocp_tiles, rcp_tiles) awaiting normalize+proj
            for qi in range(8):
                qsl = slice(qi * 512, (qi + 1) * 512)
                ps_o = [ps_op.tile([66, 512], f32, tag=f"pso{h}", name=f"pso{h}")
                        for h in range(2)]

                e3 = "e3" in ablate

                def mm1_exp(c, E2):
                    # both heads concurrently: h0 in PE rows 0-63, h1 in 64-127
                    ps = ps_sp.tile([128, 2, 512], f32, tag="pss", name="pss")
                    csl = slice(c * 128, (c + 1) * 128)
                    for h in range(2):
                        if kzpad:
                            nc.tensor.matmul(ps[:, h, :], kz[h][:, csl],
                                             q_sb[:, qsl], start=True, stop=True)
                        else:
                            hsl = slice(h * 64, (h + 1) * 64)
                            nc.tensor.matmul(ps[:, h, :], k_sb[hsl, csl],
                                             q_sb[hsl, qsl], start=True, stop=True)
                    dst = E2[:, :, c & 1, :] if e3 else E2[:, c & 1, :, :]
                    du8 = (E2.bitcast(u8)[:, :, c & 1, :] if e3
                           else E2.bitcast(u8)[:, c & 1, :, :])
                    # Bresenham-interleaved ACT/DVE split (cA of 32 on ACT)
                    if (c + 1) * cA // 32 > c * cA // 32:
                        nc.scalar.activation(dst, ps[:], AF.Exp,
                                             scale=LN2 / 8.0, bias=bias_e[:])
                    else:
                        nc.vector.tensor_scalar(du8, ps[:], B_SCH, None,
                                                op0=ALU.add)

                def mm2(cp, E2):
                    for h in range(2):
                        rhs = E2[:, h, :, :] if e3 else E2[:, :, h, :]
                        nc.tensor.matmul(ps_o[h][:],
                                         vT8[h][:, 2 * cp:2 * cp + 2, 0:66],
                                         rhs, start=(cp == 0),
                                         stop=(cp == 15), perf_mode=DR)

                Es = []
                for cp in range(16):
                    E2 = epool.tile([128, 2, 2, 512], f8, tag="E2", name="E2")
                    Es.append(E2)
                    mm1_exp(2 * cp, E2)
                    mm1_exp(2 * cp + 1, E2)
                    if cp >= 2:        # lag 2 so the qi-boundary mm2 never
                        mm2(cp - 2, Es[cp - 2])   # heads the PE FIFO early
                    if pending is not None and cp == 2:
                        normalize(*pending)
                    if pending is not None and cp == 6:
                        if "qproj" in ablate:
                            proj(pending[0])
                        pending = None
                mm2(14, Es[14])
                mm2(15, Es[15])

                # immediate epilogue: numerator + reciprocal off PSUM fast so
                # the single-buffered ps_o frees for the next qi's mm2
                ocp_t, rcp_t = [], []
                for h in range(2):
                    ocp = npool.tile([64, 512], f32r, tag="ocp", name="ocp")
                    nc.scalar.activation(ocp[:], ps_o[h][0:64, :], AF.Copy)
                    rcp = npool.tile([1, 512], f32r, tag="rcp", name="rcp")
                    nc.vector.reciprocal(rcp[:], ps_o[h][64:65, :])
                    ocp_t.append(ocp)
                    rcp_t.append(rcp)
                if "epim" in ablate:
                    normalize(qi, ocp_t, rcp_t)
                    pending = None
                else:
                    pending = (qi, ocp_t, rcp_t)
            if pending is not None:
                normalize(*pending)
            if "qproj" in ablate:
                proj(7)
            else:
                for qi in range(8):
                    proj(qi)


def _get_nc(repeats=1, ablate="", unroll=False):
    key = (repeats, ablate, unroll)
    if key not in _CACHE:
        _CACHE[key] = _build(repeats, ablate, unroll)
    return _CACHE[key]


def make_in_maps(x, gamma, beta, w_qkv, b_qkv, w_proj, b_proj):
    x = np.asarray(x, dtype=np.float32)
    gamma = np.asarray(gamma, dtype=np.float32)
    beta = np.asarray(beta, dtype=np.float32)
    w_qkv = np.asarray(w_qkv, dtype=np.float32)
    b_qkv = np.asarray(b_qkv, dtype=np.float32)
    w_proj = np.asarray(w_proj, dtype=np.float32)

    gam_in = np.ascontiguousarray(gamma.reshape(2, 128, 1))
    bet_in = np.ascontiguousarray(beta.reshape(2, 128, 1))
    sel_in = np.zeros((128, 4), dtype=np.float32)
    for g in range(4):
        sel_in[g * 32:(g + 1) * 32, g] = 1.0
    selT_in = np.ascontiguousarray(sel_in.T)
    vones_in = np.zeros((128, 32, 2), dtype=ml_dtypes.float8_e4m3)
    vones_in[:, :, 0] = 1.0 / VSCALE
    idq_in = np.zeros((128, 64), dtype=np.float32)
    idq_in[0:64] = np.eye(64, dtype=np.float32)
    idq_in[64:128] = np.eye(64, dtype=np.float32)
    in_maps = []
    for core in range(NCORES):
        b, hp = core // 2, core % 2
        rs = slice(hp * 128, (hp + 1) * 128)
        wq_s = np.concatenate([w_qkv[rs] * LOG2E,
                               w_qkv[256:][rs.start:rs.stop]], axis=0)  # [256, 256]
        wv_s = w_qkv[512:][rs.start:rs.stop]                            # [128, 256]
        # wp8[r, h, o] = w_proj[o, hp*128 + h*64 + r] * WSCALE
        wp_slice = w_proj[:, rs].T.reshape(2, 64, 256)          # [h, r, o]
        wp8 = np.ascontiguousarray(
            wp_slice.transpose(1, 0, 2) * WSCALE).astype(ml_dtypes.float8_e4m3)
        in_maps.append({
            "xb": np.ascontiguousarray(x[b].reshape(256, HW)),
            "wq": np.ascontiguousarray(wq_s.T),
            "wv": np.ascontiguousarray(wv_s.T),
            "bq": np.ascontiguousarray(
                np.stack([b_qkv[rs] * LOG2E,
                          b_qkv[512 + rs.start:512 + rs.stop]])[:, :, None]),
            "wp8": wp8,
            "gam": gam_in,
            "bet": bet_in,
            "selc": sel_in,
            "selT": selT_in,
            "vones": vones_in,
            "idq": idq_in,
        })
    return in_maps


def assemble(x, w_proj, b_proj, results):
    w_proj = np.asarray(w_proj, dtype=np.float32)
    out = np.empty((B, C, H, W), dtype=np.float32)
    scale = 1.0 / (VSCALE * WSCALE)
    for b in range(B):
        acc = (results[2 * b]["part"] + results[2 * b + 1]["part"]) * scale
        for hp in range(2):
            rs = slice(hp * 128, (hp + 1) * 128)
            bv = results[2 * b + hp]["bv"][:, 0]
            acc += (w_proj[:, rs] @ bv)[:, None]
        acc += np.asarray(b_proj, dtype=np.float32)[:, None]
        out[b] = (np.asarray(x[b], dtype=np.float32).reshape(C, HW) + acc
                  ).reshape(C, H, W)
    return out


def kernel(x, gamma, beta, w_qkv, b_qkv, w_proj, b_proj):
    from concourse.bass_utils import run_bass_kernel_spmd
    nc = _get_nc()
    in_maps = make_in_maps(x, gamma, beta, w_qkv, b_qkv, w_proj, b_proj)
    res = run_bass_kernel_spmd(nc, in_maps, core_ids=list(range(NCORES)))
    return assemble(x, w_proj, b_proj, res.results)


# revision 43
# speedup vs baseline: 1.4355x; 1.2139x over previous
"""AttnBlock (GroupNorm + 4-head hd-64 self-attention + proj + residual)
Trainium2 Bass kernel, 8 NeuronCores.

Sharding: core i handles batch b = i//2 and head-pair hp = i%2 (heads 2hp, 2hp+1).
Each core computes GroupNorm stats for its batch (folded into the QKV GEMMs as a
per-channel affine on the weights/bias), runs flash-style attention for its two
heads on-chip, and emits partial[o, pix] = sum_{c in its 128 ch} w_proj[o,c]*attn.
Host: out[b] = x[b] + b_proj + sum_hp(partial[hp]/128 + w_proj[:,hp]@bv[hp]).

Structure (the kernel is ACT/DVE-bound: every S element must exit PSUM through
one of the two engines that can read PSUM):
- mm1 (QK^T, f32r) is ROW-TILED: head0 occupies PE rows 0-63, head1 rows 64-127
  (contraction is only hd=64), so both heads' matmuls run CONCURRENTLY in the
  array -> one [128kpix, 2(head), 512q] PSUM tile per 216ns window, and the PE
  cost of mm1 halves vs zero-padded k. No kz padding tiles needed.
- V is produced PRE-TRANSPOSED by the QKV GEMM itself: lhsT = x-chunk
  (stationary), rhs = w_v^T -> out[pix, (h,hd)] accumulates in PSUM; one fp8
  exit per head per 4-chunk bank writes vT8 directly. No PE-transpose pass,
  no v_sb, half the exits.
- Softmax exp splits across ACT (true exp via table) and DVE (Schraudolph:
  round(S'+24) bit pattern as fp8e4m3, computing 2^((S'-32)/8)); log2e is
  pre-folded into the Q weights on the host so both paths are 1 op.
- Biases: K-bias dropped entirely (adds a per-query constant to S -> softmax
  invariant). V-bias returned to host (softmax weights sum to 1, so it adds
  W_proj@bv to the output). Q-bias folded into the Q-exit epilogue.
- mm2 (attn@V) and proj run fp8 DoubleRow; denominator comes out of mm2 via an
  extra 1/32 ones-column in vT8; normalize (reciprocal+broadcast+mult) is on
  DVE/gpsimd off the exit-engine critical path; proj is pipelined per-qi.
"""

import numpy as np
import ml_dtypes

B, C, H, W = 4, 256, 64, 64
HW = H * W            # 4096 pixels
NH = 4                # heads
HD = 64               # head dim
NG = 8                # groupnorm groups
EPS = 1e-5
NCORES = 8

LOG2E = 1.4426950408889634
LN2 = 0.6931471805599453
B_SCH = 24.0                      # schraudolph bias: bits = round(S' + B)
# S' = log2e*S_raw (log2e folded into Q weights).  max raw S = 62.7 ->
# S' = 90.5 -> max bits 114 < 120 (fp8e4 inf); low tail clamps to 0.
BETA_ACT = (B_SCH - 56.0) / 8.0 * LN2   # ACT path: exp(S'*ln2/8 + beta)
VSCALE = 32.0                     # denominator ones col = 1/32 -> attn x32
WSCALE = 4.0                      # w_proj stored x4
# exp engine split: per 32 chunk-blocks of a qi, how many go to ACT (rest DVE).
EXP_SPLIT = 16

_CACHE = {}


def _build(repeats=1, ablate="", unroll=False):
    import concourse.tile as tile
    from concourse import bacc, mybir

    f32 = mybir.dt.float32
    f8 = mybir.dt.float8e4

    nc = bacc.Bacc("TRN2", target_bir_lowering=False, debug=False,
                   enable_asserts=False, num_devices=NCORES)

    xb_d = nc.dram_tensor("xb", [256, HW], mybir.dt.float32r,
                          kind="ExternalInput").ap()
    # packed per-ctile weights: cols = wq(256) | wv(128) | gam | bet | bq | sel(4)
    wm_d = nc.dram_tensor("wm", [2, 128, 391], f32, kind="ExternalInput").ap()
    wp_d = nc.dram_tensor("wp8", [64, 2, 256], f8, kind="ExternalInput").ap() # [r, h, o] x4
    selT_d = nc.dram_tensor("selT", [4, 128], f32, kind="ExternalInput").ap()
    vones_d = nc.dram_tensor("vones", [128, 32, 2], f8, kind="ExternalInput").ap()
    idq_d = nc.dram_tensor("idq", [128, 64], mybir.dt.float32r,
                           kind="ExternalInput").ap()
    part_d = nc.dram_tensor("part", [256, HW], f32, kind="ExternalOutput").ap()
    bv_d = nc.dram_tensor("bv", [128, 1], f32, kind="ExternalOutput").ap()

    with tile.TileContext(nc) as tc:
        def body(_i=None):
            _body(tc, nc, mybir, xb_d, wm_d, wp_d,
                  selT_d, vones_d, idq_d, part_d, bv_d, ablate)
        if repeats == 1:
            body()
        elif unroll:
            for _ in range(repeats):
                body()
        else:
            with tc.For_i(0, repeats, 1) as _i:
                body(_i)
    nc.compile()
    return nc


def _body(tc, nc, mybir, xb_d, wm_d, wp_d,
          selT_d, vones_d, idq_d, part_d, bv_d, ablate=""):
    from contextlib import ExitStack
    AF = mybir.ActivationFunctionType
    ALU = mybir.AluOpType
    DR = mybir.MatmulPerfMode.DoubleRow
    f32 = mybir.dt.float32
    f32r = mybir.dt.float32r
    f8 = mybir.dt.float8e4
    u8 = mybir.dt.uint8
    ctx = ExitStack()
    with ctx:
        ctx.enter_context(nc.allow_low_precision("fp8/f32r attention"))
        big = ctx.enter_context(tc.tile_pool(name="big", bufs=1))
        xpool = ctx.enter_context(tc.tile_pool(name="x2", bufs=2))
        wpool = ctx.enter_context(tc.tile_pool(name="w", bufs=1))
        small = ctx.enter_context(tc.tile_pool(name="small", bufs=1))
        epool = ctx.enter_context(tc.tile_pool(name="E", bufs=4))
        npool = ctx.enter_context(tc.tile_pool(name="norm", bufs=2))

        # ---------------- load x + weights ----------------
        # small weight tensors ride the gpsimd SWDGE queue (cheap Pool-seq
        # dispatch, doesn't delay the x stream on the SP HWDGE queue); x
        # chunks split across the SP and Pool initiators.
        sdma = nc.sync if "spdma" in ablate else nc.gpsimd
        wm = []
        for t in range(2):
            wmt = wpool.tile([128, 391], f32, tag=f"wm{t}", name=f"wm{t}")
            sdma.dma_start(wmt[:], wm_d[t])
            wm.append(wmt)
        wq_raw = [wm[t][:, 0:256] for t in range(2)]
        wv_raw = [wm[t][:, 256:384] for t in range(2)]
        gam_t = [wm[t][:, 384:385] for t in range(2)]
        bet_t = [wm[t][:, 385:386] for t in range(2)]
        bq_t = [wm[blk][:, 386:387] for blk in range(2)]
        sel = wm[0][:, 387:391]
        wp8 = wpool.tile([64, 2, 256], f8, tag="wp8", name="wp8")
        sdma.dma_start(wp8[:], wp_d[:])
        selT = small.tile([4, 128], f32, tag="selT", name="selT")
        sdma.dma_start(selT[:], selT_d[:])

        xrt = []
        for t in range(2):
            xtile = xpool.tile([128, HW], f32r, tag=f"xt{t}", name=f"xt{t}")
            for ch in range(4):
                eng = nc.sync if ch % 2 == 0 else nc.scalar
                eng.dma_start(xtile[:, ch * 1024:(ch + 1) * 1024],
                              xb_d[t * 128:(t + 1) * 128,
                                   ch * 1024:(ch + 1) * 1024])
            xrt.append(xtile)
        xr = xrt
        xt = [x.bitcast(f32) for x in xrt]
        eps_t = small.tile([4, 1], f32, tag="eps", name="eps")
        nc.vector.memset(eps_t[:], EPS)
        bias_e = small.tile([128, 1], f32, tag="biasE", name="biasE")
        nc.vector.memset(bias_e[:], BETA_ACT)
        # preload the exp/ln ACT table set while the x DMA streams in
        warm = small.tile([1, 1], f32, tag="warm", name="warm")
        nc.scalar.activation(warm[:], eps_t[0:1, :], AF.Exp)
        nc.scalar.activation(warm[:], warm[:], AF.Ln)

        vT8 = [big.tile([128, 32, 96], f8, tag=f"vT{h}", name=f"vT{h}")
               for h in range(2)]
        for h in range(2):
            nc.sync.dma_start(vT8[h][:, :, 64:66], vones_d[:])

        # ---------------- groupnorm stats ----------------
        stats = []   # per tile [128, 2]: col0 mean_c, col1 E[x^2]_c
        for t in range(2):
            bno = small.tile([128, 8, 6], f32, tag=f"bno{t}", name=f"bno{t}")
            for ch in range(8):
                nc.vector.bn_stats(bno[:, ch, :], xt[t][:, ch * 512:(ch + 1) * 512])
            cst = small.tile([128, 2], f32, tag=f"cst{t}", name=f"cst{t}")
            nc.vector.bn_aggr(cst[:], bno[:])          # (mean_c, var_c)
            st = small.tile([128, 2], f32, tag=f"st{t}", name=f"st{t}")
            nc.vector.tensor_copy(st[:, 0:1], cst[:, 0:1])
            m2c = small.tile([128, 1], f32, tag=f"m2c{t}", name=f"m2c{t}")
            nc.vector.tensor_tensor(m2c[:], cst[:, 0:1], cst[:, 0:1], op=ALU.mult)
            nc.vector.tensor_tensor(st[:, 1:2], cst[:, 1:2], m2c[:], op=ALU.add)
            stats.append(st)

        with tc.tile_pool(name="ps_gn", bufs=1, space="PSUM") as ps_gn:
            psg = ps_gn.tile([4, 4], f32, tag="psg", name="psg")
            for t in range(2):
                nc.tensor.matmul(psg[:, 2 * t:2 * t + 2], sel, stats[t][:],
                                 start=True, stop=True)
            gmr = []   # per tile [4, 2]: col0 mean_g, col1 rstd_g
            for t in range(2):
                gm = small.tile([4, 2], f32, tag=f"gmr{t}", name=f"gmr{t}")
                nc.vector.tensor_scalar_mul(gm[:, 0:1], psg[:, 2 * t:2 * t + 1],
                                            1.0 / 32.0)
                m2 = small.tile([4, 1], f32, tag=f"m2{t}", name=f"m2{t}")
                nc.vector.tensor_tensor(m2[:], gm[:, 0:1], gm[:, 0:1], op=ALU.mult)
                var = small.tile([4, 1], f32, tag=f"var{t}", name=f"var{t}")
                nc.vector.scalar_tensor_tensor(var[:], psg[:, 2 * t + 1:2 * t + 2],
                                               1.0 / 32.0, m2[:],
                                               op0=ALU.mult, op1=ALU.subtract)
                lnv = small.tile([4, 1], f32, tag=f"lnv{t}", name=f"lnv{t}")
                nc.scalar.activation(lnv[:], var[:], AF.Ln, bias=eps_t[:])
                nc.scalar.activation(gm[:, 1:2], lnv[:], AF.Exp, scale=-0.5)
                gmr.append(gm)

            # per-channel scale/shift; fold into weights
            w_s, wv_s, t_r = [], [], []
            for t in range(2):
                psc = ps_gn.tile([128, 2], f32, tag="psc", name="psc")
                nc.tensor.matmul(psc[:], selT[:], gmr[t][:], start=True, stop=True)
                s_t = small.tile([128, 1], f32, tag=f"s{t}", name=f"s{t}")
                nc.vector.tensor_tensor(s_t[:], psc[:, 1:2], gam_t[t], op=ALU.mult)
                ms = small.tile([128, 1], f32, tag=f"ms{t}", name=f"ms{t}")
                nc.vector.tensor_tensor(ms[:], psc[:, 0:1], s_t[:], op=ALU.mult)
                tr = small.tile([128, 1], f32, tag=f"t{t}", name=f"t{t}")
                nc.vector.tensor_tensor(tr[:], bet_t[t], ms[:], op=ALU.subtract)
                t_r.append(tr)
                ws = wpool.tile([128, 256], f32r, tag=f"ws{t}", name=f"ws{t}")
                nc.vector.tensor_scalar_mul(ws[:], wq_raw[t], s_t[:])
                w_s.append(ws)
                wvs = wpool.tile([128, 128], f32r, tag=f"wvs{t}", name=f"wvs{t}")
                nc.vector.tensor_scalar_mul(wvs[:], wv_raw[t], s_t[:])
                wv_s.append(wvs)

            # q bias fold: b'[o] = bq[o] + sum_c Wq[o,c] * t_c   (Wq x log2e)
            # v bias:      bv[o] = bqv[o] + sum_c Wv[o,c] * t_c  -> host
            psb = ps_gn.tile([128, 2], f32, tag="psb", name="psb")
            for t in range(2):
                nc.tensor.matmul(psb[:, 0:1], wq_raw[t][:, 0:128], t_r[t][:],
                                 start=(t == 0), stop=(t == 1))
            for t in range(2):
                nc.tensor.matmul(psb[:, 1:2], wv_raw[t], t_r[t][:],
                                 start=(t == 0), stop=(t == 1))
            bias_q = small.tile([128, 1], f32, tag="biasq", name="bias_q")
            nc.vector.tensor_tensor(bias_q[:], psb[:, 0:1], bq_t[0], op=ALU.add)
            bv_sb = small.tile([128, 1], f32, tag="bvsb", name="bv_sb")
            nc.scalar.activation(bv_sb[:], psb[:, 1:2], AF.Identity,
                                 bias=bq_t[1])
            nc.sync.dma_start(bv_d[:], bv_sb[:])

        # ---------------- K/Q GEMM (f32r) ----------------
        kzpad = "kzpad" in ablate
        q_sb = big.tile([128, HW], f32r, tag="qsb", name="qsb")
        if kzpad:
            kz = [big.tile([128, HW], f32r, tag=f"kz{h}", name=f"kz{h}")
                  for h in range(2)]
            nc.gpsimd.memset(kz[0][64:128, :].bitcast(f32), 0.0)
            nc.gpsimd.memset(kz[1][0:64, :].bitcast(f32), 0.0)
            k_sb = None
        else:
            k_sb = big.tile([128, HW], f32r, tag="ksb", name="ksb")
        with tc.tile_pool(name="ps_kq", bufs=2, space="PSUM") as ps_kq:
            for blk, dst in ((1, k_sb), (0, q_sb)):       # K first
                for g in range(2):
                    ps = ps_kq.tile([128, 4, 512], f32, tag="pskq", name="pskq")
                    for j in range(4):
                        nsl = slice((4 * g + j) * 512, (4 * g + j + 1) * 512)
                        for t in range(2):
                            nc.tensor.matmul(
                                ps[:, j, :],
                                w_s[t][:, blk * 128:(blk + 1) * 128],
                                xr[t][:, nsl], start=(t == 0), stop=(t == 1))
                    gsl = slice(g * 2048, (g + 1) * 2048)
                    if blk == 1 and kzpad:   # K into zero-padded per-head tiles
                        for h in range(2):
                            hs = slice(h * 64, (h + 1) * 64)
                            kd = kz[h][hs, gsl]
                            if (g + h) % 2 == 0:
                                nc.scalar.activation(kd, ps[hs, :, :], AF.Copy)
                            else:
                                nc.vector.tensor_copy(kd, ps[hs, :, :])
                    elif blk == 1:    # K: plain copy (bias cancels in softmax)
                        if g % 2 == 0:
                            nc.scalar.activation(dst[:, gsl], ps[:], AF.Copy)
                        else:
                            nc.vector.tensor_copy(dst[:, gsl], ps[:])
                    else:             # Q: add folded bias
                        if g % 2 == 0:
                            nc.scalar.activation(dst[:, gsl], ps[:], AF.Identity,
                                                 bias=bias_q[:])
                        else:
                            nc.vector.tensor_scalar(dst[:, gsl], ps[:],
                                                    bias_q[:], None, op0=ALU.add)

            if "vold" in ablate:
                # baseline path: plain V GEMM then PE transposes
                idq = small.tile([128, 64], mybir.dt.float32r, tag="idq",
                                 name="idq")
                sdma.dma_start(idq[:], idq_d[:])
                v_sb = big.tile([128, HW], f32r, tag="vsb", name="vsb")
                for g in range(2):
                    ps = ps_kq.tile([128, 4, 512], f32, tag="pskq", name="pskq")
                    for j in range(4):
                        nsl = slice((4 * g + j) * 512, (4 * g + j + 1) * 512)
                        for t in range(2):
                            nc.tensor.matmul(
                                ps[:, j, :], wv_s[t][:],
                                xr[t][:, nsl], start=(t == 0), stop=(t == 1))
                    gsl = slice(g * 2048, (g + 1) * 2048)
                    if g % 2 == 0:
                        nc.scalar.activation(v_sb[:, gsl], ps[:], AF.Copy)
                    else:
                        nc.vector.tensor_copy(v_sb[:, gsl], ps[:])
                for h in range(2):
                    for grp in range(4):
                        pst = ps_kq.tile([128, 512], f32r, tag="pskq", name="psv2")
                        for j in range(8):
                            chunk = grp * 8 + j
                            nc.tensor.transpose(
                                pst[:, j * 64:(j + 1) * 64],
                                v_sb[h * 64:(h + 1) * 64,
                                     chunk * 128:(chunk + 1) * 128],
                                idq[h * 64:(h + 1) * 64, 0:64])
                        if (grp + h) % 2 == 0:
                            nc.scalar.activation(
                                vT8[h][:, grp * 8:(grp + 1) * 8, 0:64],
                                pst[:].rearrange("p (j d) -> p j d", d=64),
                                AF.Copy)
                        else:
                            nc.vector.tensor_copy(
                                vT8[h][:, grp * 8:(grp + 1) * 8, 0:64],
                                pst[:].rearrange("p (j d) -> p j d", d=64))
            else:
                # ------- V GEMM, pre-transposed: out[pix, (h,hd)] ----------
                for grp in range(4):
                    psV = ps_kq.tile([128, 8, 128], f32, tag="pskq", name="psV")
                    for c8 in range(8):
                        chunk = grp * 8 + c8
                        csl = slice(chunk * 128, (chunk + 1) * 128)
                        for t in range(2):
                            nc.tensor.matmul(psV[:, c8, :], xr[t][:, csl],
                                             wv_s[t][:], start=(t == 0),
                                             stop=(t == 1))
                    for h in range(2):
                        src = psV[:, :, h * 64:(h + 1) * 64]
                        dst = vT8[h][:, grp * 8:(grp + 1) * 8, 0:64]
                        if (grp + h) % 2 == 0:
                            nc.scalar.activation(dst, src, AF.Copy)
                        else:
                            nc.vector.tensor_copy(dst, src)

        # ---------------- attention ----------------
        attn8 = big.tile([64, 2, HW], f8, tag="attn8", name="attn8")
        cA = EXP_SPLIT
        import re as _re
        m = _re.search(r"cs(\d+)", ablate)
        if m:
            cA = int(m.group(1))
        with tc.tile_pool(name="ps_s", bufs=3, space="PSUM") as ps_sp, \
             tc.tile_pool(name="ps_o", bufs=1, space="PSUM") as ps_op, \
             tc.tile_pool(name="prout", bufs=2) as prout:
            def proj(qi):
                # output projection (fp8 DR); deferred into the next qi's
                # stream so the PE FIFO never waits on the gpsimd normalize.
                # psP borrows slots from the S-staging ring (PSUM is 8 banks:
                # 3x2 staging + 2 ps_o).
                qsl = slice(qi * 512, (qi + 1) * 512)
                for mch in range(2):
                    psP = ps_sp.tile([128, 512], f32, tag="pss", name="psP")
                    nc.tensor.matmul(psP[:], wp8[:, :, mch * 128:(mch + 1) * 128],
                                     attn8[:, :, qsl], start=True, stop=True,
                                     perf_mode=DR)
                    osb = prout.tile([128, 512], f32, tag="posb", name="posb")
                    if (qi + mch) % 2 == 0:
                        nc.scalar.activation(osb[:], psP[:], AF.Copy)
                    else:
                        nc.vector.tensor_copy(osb[:], psP[:])
                    nc.sync.dma_start(part_d[mch * 128:(mch + 1) * 128, qsl],
                                      osb[:])

            def normalize(qi, ocp_t, rcp_t):
                # gpsimd broadcast+mult (deferred: ~4us of Pool latency that
                # must ride under the next qi's exp stream).  For the last qi
                # the mults go on the then-idle DVE to shorten the tail.
                qsl = slice(qi * 512, (qi + 1) * 512)
                bcs = []
                for h in range(2):
                    bc = npool.tile([64, 512], f32r, tag="bc", name="bc")
                    nc.gpsimd.partition_broadcast(bc[:], rcp_t[h][:], channels=64)
                    bcs.append(bc)
                    if qi < 7:
                        nc.gpsimd.tensor_tensor(attn8[:, h, qsl], ocp_t[h][:],
                                                bc[:], op=ALU.mult)
                if qi == 7:
                    for h in range(2):
                        nc.vector.tensor_tensor(attn8[:, h, qsl], ocp_t[h][:],
                                                bcs[h][:], op=ALU.mult)

            pending = None   # (qi, ocp_tiles, rcp_tiles) awaiting normalize+proj
            for qi in range(8):
                qsl = slice(qi * 512, (qi + 1) * 512)
                ps_o = [ps_op.tile([66, 512], f32, tag=f"pso{h}", name=f"pso{h}")
                        for h in range(2)]

                def mm1_exp(c, E2):
                    # both heads concurrently: h0 in PE rows 0-63, h1 in 64-127
                    ps = ps_sp.tile([128, 2, 512], f32, tag="pss", name="pss")
                    csl = slice(c * 128, (c + 1) * 128)
                    for h in range(2):
                        hsl = slice(h * 64, (h + 1) * 64)
                        if kzpad:
                            nc.tensor.matmul(ps[:, h, :], kz[h][:, csl],
                                             q_sb[:, qsl], start=True, stop=True)
                        else:
                            nc.tensor.matmul(ps[:, h, :], k_sb[hsl, csl],
                                             q_sb[hsl, qsl], start=True, stop=True)
                    dst = E2[:, c & 1, :, :]
                    du8 = E2.bitcast(u8)[:, c & 1, :, :]
                    # Bresenham-interleaved ACT/DVE split (cA of 32 on ACT)
                    if (c + 1) * cA // 32 > c * cA // 32:
                        nc.scalar.activation(dst, ps[:], AF.Exp,
                                             scale=LN2 / 8.0, bias=bias_e[:])
                    else:
                        nc.vector.tensor_scalar(du8, ps[:], B_SCH, None,
                                                op0=ALU.add)

                def mm2(cp, E2):
                    for h in range(2):
                        nc.tensor.matmul(ps_o[h][:],
                                         vT8[h][:, 2 * cp:2 * cp + 2, 0:66],
                                         E2[:, :, h, :], start=(cp == 0),
                                         stop=(cp == 15), perf_mode=DR)

                Es = []
                for cp in range(16):
                    E2 = epool.tile([128, 2, 2, 512], f8, tag="E2", name="E2")
                    Es.append(E2)
                    mm1_exp(2 * cp, E2)
                    mm1_exp(2 * cp + 1, E2)
                    if cp >= 2:        # lag 2 so the qi-boundary mm2 never
                        mm2(cp - 2, Es[cp - 2])   # heads the PE FIFO early
                    if pending is not None and cp == 2:
                        normalize(*pending)
                        pending = None
                mm2(14, Es[14])
                mm2(15, Es[15])

                # immediate epilogue: numerator + reciprocal off PSUM fast so
                # the single-buffered ps_o frees for the next qi's mm2
                ocp_t, rcp_t = [], []
                for h in range(2):
                    ocp = npool.tile([64, 512], f32r, tag="ocp", name="ocp")
                    nc.scalar.activation(ocp[:], ps_o[h][0:64, :], AF.Copy)
                    rcp = npool.tile([1, 512], f32r, tag="rcp", name="rcp")
                    nc.vector.reciprocal(rcp[:], ps_o[h][64:65, :])
                    ocp_t.append(ocp)
                    rcp_t.append(rcp)
                if "epim" in ablate:
                    normalize(qi, ocp_t, rcp_t)
                    pending = None
                else:
                    pending = (qi, ocp_t, rcp_t)
            if pending is not None:
                normalize(*pending)
            if "qproj" in ablate:
                proj(7)
            else:
                for qi in range(8):
                    proj(qi)


def _get_nc(repeats=1, ablate="", unroll=False):
    key = (repeats, ablate, unroll)
    if key not in _CACHE:
        _CACHE[key] = _build(repeats, ablate, unroll)
    return _CACHE[key]


def make_in_maps(x, gamma, beta, w_qkv, b_qkv, w_proj, b_proj):
    x = np.asarray(x, dtype=np.float32)
    gamma = np.asarray(gamma, dtype=np.float32)
    beta = np.asarray(beta, dtype=np.float32)
    w_qkv = np.asarray(w_qkv, dtype=np.float32)
    b_qkv = np.asarray(b_qkv, dtype=np.float32)
    w_proj = np.asarray(w_proj, dtype=np.float32)

    gam_in = np.ascontiguousarray(gamma.reshape(2, 128, 1))
    bet_in = np.ascontiguousarray(beta.reshape(2, 128, 1))
    sel_in = np.zeros((128, 4), dtype=np.float32)
    for g in range(4):
        sel_in[g * 32:(g + 1) * 32, g] = 1.0
    selT_in = np.ascontiguousarray(sel_in.T)
    vones_in = np.zeros((128, 32, 2), dtype=ml_dtypes.float8_e4m3)
    vones_in[:, :, 0] = 1.0 / VSCALE
    idq_in = np.zeros((128, 64), dtype=np.float32)
    idq_in[0:64] = np.eye(64, dtype=np.float32)
    idq_in[64:128] = np.eye(64, dtype=np.float32)
    in_maps = []
    for core in range(NCORES):
        b, hp = core // 2, core % 2
        rs = slice(hp * 128, (hp + 1) * 128)
        wq_s = np.concatenate([w_qkv[rs] * LOG2E,
                               w_qkv[256:][rs.start:rs.stop]], axis=0)  # [256, 256]
        wv_s = w_qkv[512:][rs.start:rs.stop]                            # [128, 256]
        wqT = np.ascontiguousarray(wq_s.T)            # [256c, 256]
        wvT = np.ascontiguousarray(wv_s.T)            # [256c, 128]
        bqv = np.stack([b_qkv[rs] * LOG2E,
                        b_qkv[512 + rs.start:512 + rs.stop]])  # [2, 128]
        wm = np.zeros((2, 128, 391), dtype=np.float32)
        for t in range(2):
            cs = slice(t * 128, (t + 1) * 128)
            wm[t, :, 0:256] = wqT[cs]
            wm[t, :, 256:384] = wvT[cs]
            wm[t, :, 384] = gamma[cs]
            wm[t, :, 385] = beta[cs]
            wm[t, :, 386] = bqv[t]
        wm[0, :, 387:391] = sel_in
        # wp8[r, h, o] = w_proj[o, hp*128 + h*64 + r] * WSCALE
        wp_slice = w_proj[:, rs].T.reshape(2, 64, 256)          # [h, r, o]
        wp8 = np.ascontiguousarray(
            wp_slice.transpose(1, 0, 2) * WSCALE).astype(ml_dtypes.float8_e4m3)
        in_maps.append({
            "xb": np.ascontiguousarray(x[b].reshape(256, HW)),
            "wm": wm,
            "wp8": wp8,
            "selT": selT_in,
            "vones": vones_in,
            "idq": idq_in,
        })
    return in_maps


def assemble(x, w_proj, b_proj, results):
    w_proj = np.asarray(w_proj, dtype=np.float32)
    out = np.empty((B, C, H, W), dtype=np.float32)
    scale = 1.0 / (VSCALE * WSCALE)
    for b in range(B):
        acc = (results[2 * b]["part"] + results[2 * b + 1]["part"]) * scale
        for hp in range(2):
            rs = slice(hp * 128, (hp + 1) * 128)
            bv = results[2 * b + hp]["bv"][:, 0]
            acc += (w_proj[:, rs] @ bv)[:, None]
        acc += np.asarray(b_proj, dtype=np.float32)[:, None]
        out[b] = (np.asarray(x[b], dtype=np.float32).reshape(C, HW) + acc
                  ).reshape(C, H, W)
    return out


def kernel(x, gamma, beta, w_qkv, b_qkv, w_proj, b_proj):
    from concourse.bass_utils import run_bass_kernel_spmd
    nc = _get_nc()
    in_maps = make_in_maps(x, gamma, beta, w_qkv, b_qkv, w_proj, b_proj)
    res = run_bass_kernel_spmd(nc, in_maps, core_ids=list(range(NCORES)))
    return assemble(x, w_proj, b_proj, res.results)
